# revision 14
# baseline (speedup 1.0000x reference)
"""Separable box filter (radius 8) on 8 TRN2 NeuronCores via Bass/Tile.

Input  x: [8, 32, 512, 512] fp32.  Output: same shape.
Sharding: pure data parallel - batch n -> core n ([32, 512, 512] per core).

Per 512x512 (c-)slice, both 1-D box passes run as banded matmuls on the
TensorEngine, using the image data as the stationary operand (lhsT).  A
matmul computes lhsT.T @ rhs, so making the data stationary transposes
the slice; two passes restore the original orientation:

  step 1: P1[w, h'] = sum_h X[h, w] B[h, h']       (vertical box, transposed)
  step 2: out[h', w'] = sum_w P1[w, h'] B[w, w']   (horizontal box, back)

B is the 0/1 banded matrix [|i - j| <= 8]; the full 512-extent band
matrix reproduces conv2d zero padding exactly.  The whole 1/289 scale is
applied once in the final fp32 PSUM->SBUF copies, so the bf16 matmul
path only ever rounds the data, never the filter weights.

Matmuls run in bf16: weight loads get the hardware fast-weight-load path
(4x faster than fp32 - fp32 weight loads at 188 ns/matmul were the
measured bottleneck of the fp32r version of this kernel), and the
fp32->bf16 input rounding rides the SWDGE input DMA for free.

Band sparsity: contraction K-block b (rows [128b, 128b+128)) only
reaches output columns [128b - 8, 128b + 136).  The first K-block matmul
streams the full 512 columns with start=True (initializes the PSUM
bank); the remaining three stream 256-wide windows covering their
nonzero columns.
"""

import numpy as np

NCORES = 8
N_BATCH = 8
C, H, W = 32, 512, 512
R = 8
SCALE = 1.0 / float((2 * R + 1) * (2 * R + 1))

# window (start, end) of band columns streamed for K-block b = 0..3;
# block b's nonzero output columns are [128b - 8, 128b + 136).
#
# Hardware path (_WINS): all windows are 256 wide.  The first matmul of a
# bank carries start=True, which clears the whole bank's has_written bits;
# later matmuls accumulate where bits are set and overwrite where they are
# not (per-element PSUM semantics), so untouched columns get initialized
# by whichever window reaches them first.
_WINS = [(0, 256), (64, 320), (192, 448), (256, 512)]
# CoreSim asserts each matmul's output region is uniformly fresh-or-
# accumulating, so simulation uses a full-width first window instead.
_WINS_SIM = [(0, 512), (64, 320), (192, 448), (256, 512)]

_CACHE = {}


def _band_np():
    i = np.arange(H)
    band = (np.abs(i[:, None] - i[None, :]) <= R).astype(np.float32)
    return np.ascontiguousarray(band)


def _batches(c_count):
    """Graduated input-DMA batch sizes: small at both ends (fast pipeline
    fill at the start, short compute tail after the input stream ends)."""
    head = [1, 1, 2]
    tail = [2, 1, 1]
    mid_total = c_count - sum(head) - sum(tail)
    if mid_total <= 0:
        sizes = []
        for want in head + [4] * 100:
            if sum(sizes) >= c_count:
                break
            sizes.append(min(want, c_count - sum(sizes)))
        return sizes
    assert mid_total % 4 == 0, c_count
    return head + [4] * (mid_total // 4) + tail


def _build(c_count=C, sl=4, sim_safe=False):
    """Build the single-core program (same program runs SPMD on all 8)."""
    import concourse.bacc as bacc
    import concourse.mybir as mybir
    from concourse import tile

    f32 = mybir.dt.float32
    bf16 = mybir.dt.bfloat16
    act_copy = mybir.ActivationFunctionType.Copy

    nc = bacc.Bacc(trn_type="TRN2", target_bir_lowering=False, debug=False)
    x_d = nc.declare_dram_parameter("x", [c_count, H, W], f32, isOutput=False)
    band_d = nc.declare_dram_parameter("band", [H, H], f32, isOutput=False)
    out_d = nc.declare_dram_parameter("out", [c_count, H, W], f32, isOutput=True)

    wins = _WINS_SIM if sim_safe else _WINS

    with tile.TileContext(nc) as tc:
        with (
            tc.tile_pool(name="const", bufs=1) as cpool,
            tc.tile_pool(name="xin", bufs=3) as xpool,
            tc.tile_pool(name="mid", bufs=2) as mpool,
            tc.tile_pool(name="outp", bufs=3) as opool,
            tc.tile_pool(name="ps1", bufs=4, space="PSUM") as ps1,
            tc.tile_pool(name="ps2", bufs=4, space="PSUM") as ps2,
        ):
            # band matrix: 4 K-block row-tiles side by side -> [128, 4*512].
            # HWDGE fp32 load + one DVE cast, so the SWDGE queue is free to
            # start streaming the first input batch concurrently.
            band_f32 = cpool.tile([128, 4 * 512], f32, name="band_f32")
            nc.sync.dma_start(
                out=band_f32.rearrange("p (b j) -> p b j", j=512),
                in_=band_d.rearrange("(b p) j -> p b j", p=128),
            )
            band_sb = cpool.tile([128, 4 * 512], bf16, name="band_sb")
            nc.vector.tensor_copy(out=band_sb[:, :], in_=band_f32[:, :])

            c0 = 0
            for bsz in _batches(c_count):
                # one SWDGE DMA loads `bsz` slices, casting fp32 -> bf16
                xin = xpool.tile([128, bsz * 4 * 512], bf16, name="xin", tag="xin")
                nc.gpsimd.dma_start(
                    out=xin.rearrange("p (s b w) -> p s b w", s=bsz, w=512),
                    in_=x_d[c0 : c0 + bsz].rearrange("s (b p) w -> p s b w", p=128),
                )
                outsb = None
                for s in range(bsz):
                    xoff = s * 2048
                    # output staging in 2-slice groups -> 2 MB output DMAs
                    if s % 2 == 0:
                        osz = min(2, bsz - s)
                        oc0 = c0 + s
                        outsb = opool.tile(
                            [128, osz * 4 * 512], f32, name="outsb", tag="outsb"
                        )
                    ooff = (s % 2) * 2048

                    # ---- step 1: P1[w, h'] = sum_h X[h, w] B[h, h'] ----
                    p1ps = []
                    for wi in range(4):
                        p1t = ps1.tile([128, 512], f32, name="p1t", tag="p1")
                        p1ps.append(p1t)
                    for wi in range(4):
                        for hb in range(4):
                            w0, w1 = wins[hb]
                            nc.tensor.matmul(
                                p1ps[wi][:, w0:w1],
                                lhsT=xin[
                                    :,
                                    xoff + hb * 512 + wi * 128 : xoff + hb * 512 + wi * 128 + 128,
                                ],
                                rhs=band_sb[:, hb * 512 + w0 : hb * 512 + w1],
                                start=(hb == 0),
                                stop=(hb == 3),
                            )
                    # PSUM -> SBUF copies double as the fp32 -> bf16 rounding
                    p1sb = mpool.tile([128, 4 * 512], bf16, name="p1sb", tag="p1sb")
                    for wi in range(4):
                        dst = p1sb[:, wi * 512 : (wi + 1) * 512]
                        if wi < 2:
                            nc.scalar.copy(out=dst, in_=p1ps[wi][:, :])
                        else:
                            nc.vector.tensor_copy(out=dst, in_=p1ps[wi][:, :])

                    # ---- step 2: out[h', w'] = sum_w P1[w, h'] B[w, w'] ----
                    ops = []
                    for hj in range(4):
                        o_t = ps2.tile([128, 512], f32, name="o_t", tag="p2")
                        ops.append(o_t)
                    for hj in range(4):
                        for wb in range(4):
                            w0, w1 = wins[wb]
                            nc.tensor.matmul(
                                ops[hj][:, w0:w1],
                                lhsT=p1sb[
                                    :, wb * 512 + hj * 128 : wb * 512 + hj * 128 + 128
                                ],
                                rhs=band_sb[:, wb * 512 + w0 : wb * 512 + w1],
                                start=(wb == 0),
                                stop=(wb == 3),
                            )
                    # scaled PSUM -> SBUF copies apply the 1/289 factor in fp32
                    for hj in range(4):
                        dst = outsb[:, ooff + hj * 512 : ooff + (hj + 1) * 512]
                        if hj < 2:
                            nc.scalar.activation(
                                out=dst, in_=ops[hj][:, :], func=act_copy, scale=SCALE
                            )
                        else:
                            nc.vector.tensor_scalar_mul(dst, ops[hj][:, :], SCALE)

                    if s % 2 == 1 or s == bsz - 1:
                        nc.sync.dma_start(
                            out=out_d[oc0 : oc0 + osz].rearrange(
                                "s (b p) w -> p s b w", p=128
                            ),
                            in_=outsb.rearrange("p (s b w) -> p s b w", s=osz, w=512),
                        )
                c0 += bsz
    nc.compile()
    return nc


def _get_nc():
    if "nc" not in _CACHE:
        _CACHE["nc"] = _build()
    return _CACHE["nc"]


def _run(x, trace=False, tmpdir=None):
    """Run on 8 cores; returns (out [8,32,512,512], exec_time_ns or None)."""
    from concourse.bass_utils import run_bass_kernel_spmd

    x = np.ascontiguousarray(np.asarray(x, dtype=np.float32))
    assert x.shape == (N_BATCH, C, H, W), x.shape
    band = _band_np()
    nc = _get_nc()
    in_maps = [{"x": x[i], "band": band} for i in range(NCORES)]
    res = run_bass_kernel_spmd(
        nc, in_maps, core_ids=list(range(NCORES)), trace=trace, tmpdir=tmpdir
    )
    out = np.stack(
        [res.results[i]["out"] for i in range(NCORES)], axis=0
    ).astype(np.float32)
    return out, res.exec_time_ns


def kernel(x):
    out, _ = _run(x)
    return out


# revision 16
# speedup vs baseline: 1.0397x; 1.0397x over previous
"""Separable box filter (radius 8) on 8 TRN2 NeuronCores via Bass/Tile.

Input  x: [8, 32, 512, 512] fp32.  Output: same shape.
Sharding: pure data parallel - batch n -> core n ([32, 512, 512] per core).

Per 512x512 (c-)slice, both 1-D box passes run as banded matmuls on the
TensorEngine, using the image data as the stationary operand (lhsT).  A
matmul computes lhsT.T @ rhs, so making the data stationary transposes
the slice; two passes restore the original orientation:

  step 1: P1[w, h'] = sum_h X[h, w] B[h, h']       (vertical box, transposed)
  step 2: out[h', w'] = sum_w P1[w, h'] B[w, w']   (horizontal box, back)

B is the 0/1 banded matrix [|i - j| <= 8]; the full 512-extent band
matrix reproduces conv2d zero padding exactly.  The whole 1/289 scale is
applied once in the final fp32 PSUM->SBUF copies, so the bf16 matmul
path only ever rounds the data, never the filter weights.

Matmuls run in bf16: weight loads get the hardware fast-weight-load path
(4x faster than fp32 - fp32 weight loads at 188 ns/matmul were the
measured bottleneck of the fp32r version of this kernel), and the
fp32->bf16 input rounding rides the SWDGE input DMA for free.

Band sparsity: contraction K-block b (rows [128b, 128b+128)) only
reaches output columns [128b - 8, 128b + 136).  The first K-block matmul
streams the full 512 columns with start=True (initializes the PSUM
bank); the remaining three stream 256-wide windows covering their
nonzero columns.
"""

import numpy as np

NCORES = 8
N_BATCH = 8
C, H, W = 32, 512, 512
R = 8
SCALE = 1.0 / float((2 * R + 1) * (2 * R + 1))

# window (start, end) of band columns streamed for K-block b = 0..3;
# block b's nonzero output columns are [128b - 8, 128b + 136).
#
# Hardware path (_WINS): all windows are 256 wide.  The first matmul of a
# bank carries start=True, which clears the whole bank's has_written bits;
# later matmuls accumulate where bits are set and overwrite where they are
# not (per-element PSUM semantics), so untouched columns get initialized
# by whichever window reaches them first.
_WINS = [(0, 256), (64, 320), (192, 448), (256, 512)]
# CoreSim asserts each matmul's output region is uniformly fresh-or-
# accumulating, so simulation uses a full-width first window instead.
_WINS_SIM = [(0, 512), (64, 320), (192, 448), (256, 512)]

_CACHE = {}


def _band_np():
    i = np.arange(H)
    band = (np.abs(i[:, None] - i[None, :]) <= R).astype(np.float32)
    return np.ascontiguousarray(band)


def _batches(c_count):
    """Graduated input-DMA batch sizes: small first (fast pipeline fill)."""
    sizes = []
    for want in [1, 1, 2] + [4] * 100:
        if sum(sizes) >= c_count:
            break
        sizes.append(min(want, c_count - sum(sizes)))
    return sizes


def _build(c_count=C, sl=4, sim_safe=False):
    """Build the single-core program (same program runs SPMD on all 8)."""
    import concourse.bacc as bacc
    import concourse.mybir as mybir
    from concourse import tile

    f32 = mybir.dt.float32
    bf16 = mybir.dt.bfloat16
    act_copy = mybir.ActivationFunctionType.Copy

    nc = bacc.Bacc(trn_type="TRN2", target_bir_lowering=False, debug=False)
    x_d = nc.declare_dram_parameter("x", [c_count, H, W], f32, isOutput=False)
    band_d = nc.declare_dram_parameter("band", [H, H], f32, isOutput=False)
    out_d = nc.declare_dram_parameter("out", [c_count, H, W], f32, isOutput=True)

    wins = _WINS_SIM if sim_safe else _WINS

    with tile.TileContext(nc) as tc:
        with (
            tc.tile_pool(name="const", bufs=1) as cpool,
            tc.tile_pool(name="xin", bufs=4) as xpool,
            tc.tile_pool(name="mid", bufs=2) as mpool,
            tc.tile_pool(name="outp", bufs=3) as opool,
            tc.tile_pool(name="ps1", bufs=4, space="PSUM") as ps1,
            tc.tile_pool(name="ps2", bufs=4, space="PSUM") as ps2,
        ):
            # band matrix: 4 K-block row-tiles side by side -> [128, 4*512].
            # HWDGE fp32 load + one DVE cast, so the SWDGE queue is free to
            # start streaming the first input batch concurrently.
            band_f32 = cpool.tile([128, 4 * 512], f32, name="band_f32")
            nc.sync.dma_start(
                out=band_f32.rearrange("p (b j) -> p b j", j=512),
                in_=band_d.rearrange("(b p) j -> p b j", p=128),
            )
            band_sb = cpool.tile([128, 4 * 512], bf16, name="band_sb")
            nc.vector.tensor_copy(out=band_sb[:, :], in_=band_f32[:, :])

            c0 = 0
            for bsz in _batches(c_count):
                # one SWDGE DMA loads `bsz` slices, casting fp32 -> bf16
                xin = xpool.tile([128, bsz * 4 * 512], bf16, name="xin", tag="xin")
                nc.gpsimd.dma_start(
                    out=xin.rearrange("p (s b w) -> p s b w", s=bsz, w=512),
                    in_=x_d[c0 : c0 + bsz].rearrange("s (b p) w -> p s b w", p=128),
                )
                outsb = None
                for s in range(bsz):
                    xoff = s * 2048
                    # output staging in 2-slice groups -> 2 MB output DMAs
                    if s % 2 == 0:
                        osz = min(2, bsz - s)
                        oc0 = c0 + s
                        outsb = opool.tile(
                            [128, osz * 4 * 512], f32, name="outsb", tag="outsb"
                        )
                    ooff = (s % 2) * 2048

                    # ---- step 1: P1[w, h'] = sum_h X[h, w] B[h, h'] ----
                    p1ps = []
                    for wi in range(4):
                        p1t = ps1.tile([128, 512], f32, name="p1t", tag="p1")
                        p1ps.append(p1t)
                    for wi in range(4):
                        for hb in range(4):
                            w0, w1 = wins[hb]
                            nc.tensor.matmul(
                                p1ps[wi][:, w0:w1],
                                lhsT=xin[
                                    :,
                                    xoff + hb * 512 + wi * 128 : xoff + hb * 512 + wi * 128 + 128,
                                ],
                                rhs=band_sb[:, hb * 512 + w0 : hb * 512 + w1],
                                start=(hb == 0),
                                stop=(hb == 3),
                            )
                    # PSUM -> SBUF copies double as the fp32 -> bf16 rounding
                    p1sb = mpool.tile([128, 4 * 512], bf16, name="p1sb", tag="p1sb")
                    for wi in range(4):
                        dst = p1sb[:, wi * 512 : (wi + 1) * 512]
                        if wi < 2:
                            nc.scalar.copy(out=dst, in_=p1ps[wi][:, :])
                        else:
                            nc.vector.tensor_copy(out=dst, in_=p1ps[wi][:, :])

                    # ---- step 2: out[h', w'] = sum_w P1[w, h'] B[w, w'] ----
                    ops = []
                    for hj in range(4):
                        o_t = ps2.tile([128, 512], f32, name="o_t", tag="p2")
                        ops.append(o_t)
                    for hj in range(4):
                        for wb in range(4):
                            w0, w1 = wins[wb]
                            nc.tensor.matmul(
                                ops[hj][:, w0:w1],
                                lhsT=p1sb[
                                    :, wb * 512 + hj * 128 : wb * 512 + hj * 128 + 128
                                ],
                                rhs=band_sb[:, wb * 512 + w0 : wb * 512 + w1],
                                start=(wb == 0),
                                stop=(wb == 3),
                            )
                    # scaled PSUM -> SBUF copies apply the 1/289 factor in fp32
                    for hj in range(4):
                        dst = outsb[:, ooff + hj * 512 : ooff + (hj + 1) * 512]
                        if hj < 2:
                            nc.scalar.activation(
                                out=dst, in_=ops[hj][:, :], func=act_copy, scale=SCALE
                            )
                        else:
                            nc.vector.tensor_scalar_mul(dst, ops[hj][:, :], SCALE)

                    if s % 2 == 1 or s == bsz - 1:
                        nc.sync.dma_start(
                            out=out_d[oc0 : oc0 + osz].rearrange(
                                "s (b p) w -> p s b w", p=128
                            ),
                            in_=outsb.rearrange("p (s b w) -> p s b w", s=osz, w=512),
                        )
                c0 += bsz
    nc.compile()
    return nc


def _get_nc():
    if "nc" not in _CACHE:
        _CACHE["nc"] = _build()
    return _CACHE["nc"]


def _run(x, trace=False, tmpdir=None):
    """Run on 8 cores; returns (out [8,32,512,512], exec_time_ns or None)."""
    from concourse.bass_utils import run_bass_kernel_spmd

    x = np.ascontiguousarray(np.asarray(x, dtype=np.float32))
    assert x.shape == (N_BATCH, C, H, W), x.shape
    band = _band_np()
    nc = _get_nc()
    in_maps = [{"x": x[i], "band": band} for i in range(NCORES)]
    res = run_bass_kernel_spmd(
        nc, in_maps, core_ids=list(range(NCORES)), trace=trace, tmpdir=tmpdir
    )
    out = np.stack(
        [res.results[i]["out"] for i in range(NCORES)], axis=0
    ).astype(np.float32)
    return out, res.exec_time_ns


def kernel(x):
    out, _ = _run(x)
    return out
